# revision 1
# baseline (speedup 1.0000x reference)
"""Trainium2 Bass kernel for DFlashAttentionV5.

Reference computation (fp32, single device):
    Q/K/V/Kctx/Vctx projections -> rmsnorm(Q), rmsnorm(K_full) -> softmax
    attention over concat(ctx, self) keys/values -> output projection.

Sharding over 8 NeuronCores: batch (2-way) x head-group (4-way).
Core c handles batch b = c // 4 and heads 4*g..4*g+3 where g = c % 4.
Each core computes attention for its 4 local heads; after each head it
all-gathers that head's attention output (transposed layout [hd, tokens])
across its 4-core batch group and immediately accumulates that head-wave
of the output projection for its 512-wide output column slice, so the
collectives and the output projection overlap the remaining heads.

All matmuls run in bf16 (fp32 PSUM accumulation); softmax statistics and
normalization factors are computed in fp32. x/ctx arrive pre-transposed
from the host ([D, tokens]) so no on-device transposes are needed.

Self-contained: hardcodes all shapes; only imports concourse + numpy.
"""

import math

import numpy as np
import ml_dtypes

import concourse.bass as bass
import concourse.mybir as mybir
import concourse.tile as tile
from concourse.bass_utils import run_bass_kernel_spmd

BF16 = mybir.dt.bfloat16
F32 = mybir.dt.float32
AF = mybir.ActivationFunctionType
ALU = mybir.AluOpType

# Problem dims
B, K, CTX, D, H, HD = 2, 1024, 2048, 2048, 16, 128
S = CTX + K            # 3072 keys per query
NCORES = 8
GROUPS = 4             # head groups (tensor-parallel within a batch)
NH = H // GROUPS       # 4 local heads per core
E = H * HD             # 2048
EW = NH * HD           # 512 local attention width / weight shard width
DCH = D // 128         # 16 contraction chunks
SCH = S // 128         # 24 key chunks
TCH = K // 128         # 8 query-token chunks
SCALE = 1.0 / math.sqrt(HD)
EPS = 1e-6
REPLICA_GROUPS = [[0, 1, 2, 3], [4, 5, 6, 7]]

_CACHE = {}


def _build(with_mask: bool):
    """Build the SPMD bass program (same program on all 8 cores)."""
    nc = bass.Bass(num_devices=NCORES)

    xT_d = nc.declare_dram_parameter("xT", [D, K], BF16, isOutput=False)
    cT_d = nc.declare_dram_parameter("cT", [D, CTX], BF16, isOutput=False)
    wq_d = nc.declare_dram_parameter("wq", [D, EW], BF16, isOutput=False)
    wk_d = nc.declare_dram_parameter("wk", [D, EW], BF16, isOutput=False)
    wv_d = nc.declare_dram_parameter("wv", [D, EW], BF16, isOutput=False)
    wck_d = nc.declare_dram_parameter("wck", [D, EW], BF16, isOutput=False)
    wcv_d = nc.declare_dram_parameter("wcv", [D, EW], BF16, isOutput=False)
    wo_d = nc.declare_dram_parameter("wo", [E, EW], BF16, isOutput=False)
    qnw_d = nc.declare_dram_parameter("qnw", [HD, 1], F32, isOutput=False)
    knw_d = nc.declare_dram_parameter("knw", [HD, 1], F32, isOutput=False)
    if with_mask:
        mt_d = nc.declare_dram_parameter("maskT", [S, K], F32, isOutput=False)
    out_d = nc.declare_dram_parameter("out", [K, EW], F32, isOutput=True)

    with tile.TileContext(nc, num_cores=NCORES) as tc:
        with (
            tc.tile_pool(name="const", bufs=1) as constp,
            tc.tile_pool(name="perm", bufs=1) as perm,
            tc.tile_pool(name="stat", bufs=2) as statp,
            tc.tile_pool(name="bc", bufs=2) as bcp,
            tc.tile_pool(name="psA", bufs=3, space="PSUM") as psA,
            tc.tile_pool(name="ps1", bufs=2, space="PSUM") as ps1,
            tc.tile_pool(name="dram", bufs=1, space="DRAM") as dram,
        ):
            ones_col = constp.tile([128, 1], BF16)
            nc.any.memset(ones_col, 1.0)
            ones_row = constp.tile([1, 128], BF16)
            nc.any.memset(ones_row, 1.0)
            qnw_sb = constp.tile([HD, 1], F32)
            knw_sb = constp.tile([HD, 1], F32)

            # Resident tensors (bf16):
            #   K_sb[h]  [128=hd, 3072=s] per local head (ctx keys then self)
            #   V_sb[s]  [128=s-chunk, 512=4 heads x hd], s 0..15 ctx, 16..23 self
            #   QT_sb[h] [128=hd, 1024=q]
            K_sb = [perm.tile([128, S], BF16, tag=f"K{h}", bufs=1, name=f"K{h}")
                    for h in range(NH)]
            V_sb = [perm.tile([128, EW], BF16, tag=f"V{s}", bufs=1, name=f"V{s}")
                    for s in range(SCH)]
            QT_sb = [perm.tile([128, K], BF16, tag=f"Q{h}", bufs=1, name=f"Q{h}")
                     for h in range(NH)]

            attnT_loc = [dram.tile([128, K], BF16, name=f"atl{h}")
                         for h in range(NH)]
            attnT_gat = [dram.tile([GROUPS * 128, K], BF16, name=f"atg{h}")
                         for h in range(NH)]

            # ---- helper: rmsnorm in transposed layout.
            # ps [128=hd, width=tokens] fp32 PSUM -> dest bf16 SBUF.
            # norm over hd (partitions): mean of squares via ones-matmul,
            # rsqrt via reciprocal+sqrt, broadcast across partitions via K=1
            # matmul, apply with one scalar_tensor_tensor (folds norm weight).
            def rms_norm_T(sqp, ps, dest_ap, width, nw_sb):
                # norm over hd (partitions): mean of squares via ones-matmul,
                # rsqrt via reciprocal+sqrt, broadcast across partitions via
                # K=1 matmul, applied with one scalar_tensor_tensor (also
                # folds the norm weight).
                sqt = sqp.tile([128, 1024], BF16, tag="sq")
                nc.scalar.square(sqt[:, :width], ps[:, :width])
                for j in range(width // 512):
                    js = slice(j * 512, (j + 1) * 512)
                    ps_s = ps1.tile([128, 512], F32, tag="ps1")
                    nc.tensor.matmul(ps_s[0:1, :], ones_col[:], sqt[:, js],
                                     start=True, stop=True)
                    mean = statp.tile([1, 512], F32, tag="mean")
                    nc.vector.tensor_scalar(mean[:], ps_s[0:1, :], 1.0 / HD, EPS,
                                            ALU.mult, ALU.add)
                    rec = statp.tile([1, 512], F32, tag="rec")
                    nc.vector.reciprocal(rec[:], mean[:])
                    rs = statp.tile([1, 512], BF16, tag="rs")
                    nc.scalar.sqrt(rs[:], rec[:])  # rsqrt = sqrt(1/x), bf16
                    ps_b = ps1.tile([128, 512], F32, tag="ps1")
                    nc.tensor.matmul(ps_b[:], ones_row[:], rs[:],
                                     start=True, stop=True)
                    bc = bcp.tile([128, 512], F32, tag="bc")
                    nc.scalar.copy(bc[:], ps_b[:])
                    nc.vector.scalar_tensor_tensor(
                        dest_ap[:, js], ps[:, js], nw_sb[:], bc[:],
                        ALU.mult, ALU.mult)

            # ================= projection phase =================
            with (
                tc.tile_pool(name="srcT", bufs=1) as srcTp,
                tc.tile_pool(name="wstream", bufs=4) as wstream,
                tc.tile_pool(name="wwide", bufs=1) as wwide,
                tc.tile_pool(name="sqp", bufs=2) as sqp,
            ):
                # d-chunk accessor over grouped source tiles [128, 4*1024]
                def src_at(grp, d):
                    return grp[d // 4], (d % 4) * 1024

                def load_set(grp, dram_ap, split_first=False):
                    # dram_ap: [D, 1024] (d-major); one DMA per 4 d-chunks.
                    # split_first peels d-chunk 0 into its own small DMA so
                    # the first dependent matmul can start sooner.
                    if split_first:
                        nc.sync.dma_start(
                            grp[0][:, 0:1024],
                            dram_ap[0:128, :])
                        nc.sync.dma_start(
                            grp[0][:, 1024:4096].rearrange("p (a t) -> p a t",
                                                           t=1024),
                            dram_ap[128:512, :]
                            .rearrange("(a p) t -> p a t", p=128))
                    else:
                        nc.sync.dma_start(
                            grp[0][:].rearrange("p (a t) -> p a t", t=1024),
                            dram_ap[0:512, :]
                            .rearrange("(a p) t -> p a t", p=128))
                    for i in range(1, 4):
                        nc.sync.dma_start(
                            grp[i][:].rearrange("p (a t) -> p a t", t=1024),
                            dram_ap[i * 512:(i + 1) * 512, :]
                            .rearrange("(a p) t -> p a t", p=128))

                # Q^T / K^T projections (weight-stationary):
                # psum[c] [128=col-chunk(head), 1024 tokens] += w[d,c].T @ srcT[d]
                def load_wchunk(w_d, c, name=None):
                    wch = wstream.tile([128, D], BF16, tag="w", name=name)
                    nc.sync.dma_start(
                        wch[:].rearrange("p (a q) -> p a q", q=128),
                        w_d[:, c * 128:(c + 1) * 128]
                        .rearrange("(a p) q -> p a q", p=128))
                    return wch

                def qk_proj(w_d, srcT, dest_of_chunk, nw_sb, pre=None):
                    for c in range(EW // 128):
                        wch = pre if (c == 0 and pre is not None) \
                            else load_wchunk(w_d, c)
                        ps = psA.tile([128, 1024], F32, tag="psA",
                                      name=f"psqk{c}")
                        for d in range(DCH):
                            st, off = src_at(srcT, d)
                            for j in range(2):
                                nc.tensor.matmul(
                                    ps[:, j * 512:(j + 1) * 512],
                                    wch[:, d * 128:(d + 1) * 128],
                                    st[:, off + j * 512:off + j * 512 + 512],
                                    start=(d == 0), stop=(d == DCH - 1))
                        dest, off = dest_of_chunk(c)
                        rms_norm_T(sqp, ps, dest[:, off:off + 1024], 1024, nw_sb)

                # V projections (activation-stationary):
                # V_sb[s] [128=tokens, 512=cols] += srcT[d][:,t-chunk].T @ wv[d]
                def load_wide(w_d, tag):
                    grp = []
                    for i in range(4):
                        wt = wwide.tile([128, 4 * EW], BF16, tag=f"wv{i}",
                                        bufs=1, name=f"wv{i}_{tag}")
                        nc.sync.dma_start(
                            wt[:].rearrange("p (a q) -> p a q", q=EW),
                            w_d[i * 512:(i + 1) * 512, :]
                            .rearrange("(a p) q -> p a q", p=128))
                        grp.append(wt)
                    return grp

                def v_proj(wv_grp, srcT, s_base):
                    for t in range(TCH):
                        ps = ps1.tile([128, 512], F32, tag="ps1")
                        for d in range(DCH):
                            st, off = src_at(srcT, d)
                            wvt = wv_grp[d // 4]
                            wo_off = (d % 4) * EW
                            nc.tensor.matmul(
                                ps[:], st[:, off + t * 128:off + (t + 1) * 128],
                                wvt[:, wo_off:wo_off + EW],
                                start=(d == 0), stop=(d == DCH - 1))
                        nc.vector.tensor_copy(V_sb[s_base + t][:], ps[:])

                # slot sets: A = xT then ctx-half1 (recycled), B = ctx-half0
                setA = [srcTp.tile([128, 4096], BF16, tag=f"sa{i}", bufs=1,
                                   name=f"xT{i}") for i in range(4)]
                setB = [srcTp.tile([128, 4096], BF16, tag=f"sb{i}", bufs=1,
                                   name=f"cTa{i}") for i in range(4)]
                pre_wq = load_wchunk(wq_d, 0, name="prewq")
                load_set(setA, xT_d, split_first=True)
                nc.sync.dma_start(qnw_sb[:], qnw_d[:])
                nc.sync.dma_start(knw_sb[:], knw_d[:])

                # self tokens (block A): Q, K_self, V_self
                qk_proj(wq_d, setA, lambda c: (QT_sb[c], 0), qnw_sb, pre=pre_wq)
                load_set(setB, cT_d[:, 0:1024])
                wv_sb = load_wide(wv_d, "s")
                qk_proj(wk_d, setA, lambda c: (K_sb[c], CTX), knw_sb)
                v_proj(wv_sb, setA, CTX // 128)

                # ctx half 0 (block B): K_ctx[:, 0:1024], V_ctx s-chunks 0..7
                wcv_sb = load_wide(wcv_d, "c0")
                qk_proj(wck_d, setB, lambda c: (K_sb[c], 0), knw_sb)
                v_proj(wcv_sb, setB, 0)

                # ctx half 1 reuses set A slots
                setC = [srcTp.tile([128, 4096], BF16, tag=f"sa{i}", bufs=1,
                                   name=f"cTb{i}") for i in range(4)]
                load_set(setC, cT_d[:, 1024:2048])
                wcv2_sb = load_wide(wcv_d, "c1")
                qk_proj(wck_d, setC, lambda c: (K_sb[c], 1024), knw_sb)
                v_proj(wcv2_sb, setC, TCH)

            # ================= attention + output phase =================
            with (
                tc.tile_pool(name="probsT", bufs=8) as probsp,
                tc.tile_pool(name="dacc", bufs=2) as daccp,
                tc.tile_pool(name="accb", bufs=2) as accbp,
                tc.tile_pool(name="attnTp", bufs=2) as attnTp,
                tc.tile_pool(name="wop", bufs=1) as wop,
                tc.tile_pool(name="atile", bufs=8) as atilep,
                tc.tile_pool(name="oacc", bufs=1) as oaccp,
                tc.tile_pool(name="mrow", bufs=4) as mrowp,
            ):
                wo_grp = [wop.tile([128, 4 * EW], BF16, tag=f"wo{i}", bufs=1,
                                   name=f"wo{i}") for i in range(4)]
                for i in range(4):
                    nc.sync.dma_start(
                        wo_grp[i][:].rearrange("p (a q) -> p a q", q=EW),
                        wo_d[i * 512:(i + 1) * 512, :]
                        .rearrange("(a p) q -> p a q", p=128))

                def wo_at(e):
                    return wo_grp[e // 4][:, (e % 4) * EW:(e % 4 + 1) * EW]
                out_acc = [oaccp.tile([128, EW], F32, tag=f"oa{t}", bufs=1,
                                      name=f"oa{t}") for t in range(TCH)]

                # attention for one local head, transposed scores:
                # scoresT[s-chunk] [128=s, 1024=q] = K_chunk @ Q^T  (no max
                # subtraction: scores ~ N(0,1) after rmsnorm + 1/sqrt(HD))
                def attention(h, q0=0, qw=K, dst=None):
                    # processes queries [q0, q0+qw) for local head h
                    nj = qw // 512
                    ps_pv = psA.tile([128, 1024], F32, tag="psA", name=f"pv{h}_{q0}")
                    acc = daccp.tile([128, 1024], F32, tag="dacc", name=f"dac{h}")
                    for s in range(SCH):
                        ps_sT = psA.tile([128, 1024], F32, tag="psA",
                                         name=f"sT{h}_{q0}_{s}")
                        for j in range(nj):
                            nc.tensor.matmul(
                                ps_sT[:, j * 512:(j + 1) * 512],
                                K_sb[h][:, s * 128:(s + 1) * 128],
                                QT_sb[h][:, q0 + j * 512:q0 + (j + 1) * 512],
                                start=True, stop=True)
                        if with_mask:
                            mrow = mrowp.tile([128, K], F32, tag="mrow")
                            nc.sync.dma_start(
                                mrow[:, :qw],
                                mt_d[s * 128:(s + 1) * 128, q0:q0 + qw])
                            nc.vector.tensor_tensor(ps_sT[:, :qw], ps_sT[:, :qw],
                                                    mrow[:, :qw], ALU.add)
                        pT = probsp.tile([128, 1024], BF16, tag="pT")
                        nc.scalar.activation(pT[:, :qw], ps_sT[:, :qw], AF.Exp,
                                             scale=SCALE)
                        first, last = (s == 0), (s == SCH - 1)
                        # probs accumulate on the DVE (f32) for the softmax
                        # denominators; the PE only does scores and PV
                        if first:
                            nc.vector.tensor_copy(acc[:, :qw], pT[:, :qw])
                        else:
                            nc.vector.tensor_tensor(acc[:, :qw], acc[:, :qw],
                                                    pT[:, :qw], ALU.add)
                        for j in range(nj):
                            js = slice(j * 512, (j + 1) * 512)
                            nc.tensor.matmul(
                                ps_pv[:, js],
                                V_sb[s][:, h * 128:(h + 1) * 128], pT[:, js],
                                start=first, stop=last)
                    # normalize: attnT = ps_pv * (1/denom), denom broadcast
                    # across partitions via K=1 matmul
                    at = attnTp.tile([128, K], BF16, tag="at")
                    accb = accbp.tile([128, 1024], BF16, tag="accb")
                    nc.vector.tensor_copy(accb[:, :qw], acc[:, :qw])
                    for j in range(nj):
                        js = slice(j * 512, (j + 1) * 512)
                        ps_d = ps1.tile([128, 512], F32, tag="ps1")
                        nc.tensor.matmul(ps_d[0:1, :], ones_col[:], accb[:, js],
                                         start=True, stop=True)
                        rec = statp.tile([1, 512], F32, tag="rec")
                        nc.vector.reciprocal(rec[:], ps_d[0:1, :])
                        rb = statp.tile([1, 512], BF16, tag="rb")
                        nc.vector.tensor_copy(rb[:], rec[:])
                        ps_b = ps1.tile([128, 512], F32, tag="ps1")
                        nc.tensor.matmul(ps_b[:], ones_row[:], rb[:],
                                         start=True, stop=True)
                        bc = bcp.tile([128, 512], F32, tag="bc")
                        nc.scalar.copy(bc[:], ps_b[:])
                        nc.vector.tensor_tensor(at[:, js], ps_pv[:, js], bc[:],
                                                ALU.mult)
                    dst_ap = dst if dst is not None else attnT_loc[h][:]
                    nc.sync.dma_start(dst_ap, at[:, :qw])

                # output projection wave for gathered head h:
                # out_acc[t] += attnT_gat[h][g-chunk, t-chunk].T @ wo[g*4+h]
                def out_wave(h, src=None, t0=0, tn=TCH):
                    gat = src if src is not None else attnT_gat[h]
                    for t in range(t0, t0 + tn):
                        ps = ps1.tile([128, 512], F32, tag="ps1")
                        at4 = atilep.tile([128, 512], BF16, tag="at")
                        nc.sync.dma_start(
                            at4[:].rearrange("p (g q) -> p g q", q=128),
                            gat[:, (t - t0) * 128:(t - t0 + 1) * 128]
                            .rearrange("(g p) q -> p g q", p=128))
                        for g in range(GROUPS):
                            nc.tensor.matmul(ps[:],
                                             at4[:, g * 128:(g + 1) * 128],
                                             wo_at(g * NH + h),
                                             start=(g == 0),
                                             stop=(g == GROUPS - 1))
                        if h == 0:
                            nc.vector.tensor_copy(out_acc[t][:], ps[:])
                        else:
                            nc.vector.tensor_tensor(out_acc[t][:], out_acc[t][:],
                                                    ps[:], ALU.add)

                # software pipeline: gather h overlaps attention h+1; the
                # out-projection wave for head h is emitted after attention
                # h+1 so the PE stream never waits on an in-flight gather.
                def gather(in_t, out_t):
                    nc.gpsimd.collective_compute(
                        "AllGather", ALU.bypass,
                        replica_groups=REPLICA_GROUPS,
                        ins=[in_t.opt()],
                        outs=[out_t.opt()],
                    )

                # attentions run back-to-back so every gather starts as
                # early as possible (the gather chain is the critical path);
                # the out-projection waves fill the tail while the last
                # gathers drain.
                for h in range(NH):
                    attention(h)
                    gather(attnT_loc[h], attnT_gat[h])
                for h in range(NH):
                    out_wave(h)

                for t in range(TCH):
                    nc.sync.dma_start(out_d[t * 128:(t + 1) * 128, :],
                                      out_acc[t][:])

    return nc


def _split_multiwaits(nc):
    """walrus codegen in this container rejects instructions with more than
    one semaphore wait; split the excess onto preceding NoOps on the same
    engine."""
    for f in nc.m.functions:
        for blk in f.blocks:
            idx = 0
            while idx < len(blk.instructions):
                inst = blk.instructions[idx]
                si = inst.sync_info
                maxw = 1
                if si is None or len(si.on_wait) <= maxw:
                    idx += 1
                    continue
                waits = list(si.on_wait)
                ncarry = (len(waits) - 1) // maxw  # leave <=maxw on inst
                for k in range(ncarry):
                    chunk = waits[k * maxw:(k + 1) * maxw]
                    nop = mybir.InstNoOp(
                        name=nc.get_next_instruction_name(),
                        ins=[], outs=[],
                        bass_nofuse=True,
                        sync_info=mybir.SyncInfo(on_wait=chunk, on_update=[]),
                    )
                    nop.engine = inst.engine
                    nc.register_instruction(nop)
                    blk.instructions.insert(idx, nop)
                    idx += 1
                si.on_wait = waits[ncarry * maxw:]
                idx += 1


def _get_program(with_mask: bool):
    key = ("prog", with_mask)
    if key not in _CACHE:
        nc = _build(with_mask)
        _split_multiwaits(nc)
        _CACHE[key] = nc
    return _CACHE[key]


def kernel(x, context, attn_mask, w_q, w_k, w_v, w_ctx_k, w_ctx_v, w_out,
           q_norm_w, k_norm_w):
    x = np.asarray(x, np.float32)
    context = np.asarray(context, np.float32)
    attn_mask = np.asarray(attn_mask, np.float32)
    w_q = np.asarray(w_q, np.float32)
    w_k = np.asarray(w_k, np.float32)
    w_v = np.asarray(w_v, np.float32)
    w_ctx_k = np.asarray(w_ctx_k, np.float32)
    w_ctx_v = np.asarray(w_ctx_v, np.float32)
    w_out = np.asarray(w_out, np.float32)
    q_norm_w = np.asarray(q_norm_w, np.float32)
    k_norm_w = np.asarray(k_norm_w, np.float32)

    with_mask = bool(np.any(attn_mask))
    nc = _get_program(with_mask)
    in_maps = _prepare_in_maps(x, context, attn_mask, w_q, w_k, w_v, w_ctx_k,
                               w_ctx_v, w_out, q_norm_w, k_norm_w, with_mask)

    res = run_bass_kernel_spmd(nc, in_maps, list(range(NCORES))).results
    return _assemble(res)


def _assemble(res):
    out = np.empty((B, K, D), np.float32)
    for c in range(NCORES):
        b, g = c // GROUPS, c % GROUPS
        out[b, :, g * EW:(g + 1) * EW] = res[c]["out"]
    return out


def _prepare_in_maps(x, context, attn_mask, w_q, w_k, w_v, w_ctx_k, w_ctx_v,
                     w_out, q_norm_w, k_norm_w, with_mask):
    bf16 = ml_dtypes.bfloat16
    xT = [np.ascontiguousarray(x[b].T).astype(bf16) for b in range(B)]
    cT = [np.ascontiguousarray(context[b].T).astype(bf16) for b in range(B)]
    in_maps = []
    for c in range(NCORES):
        b, g = c // GROUPS, c % GROUPS
        cols = slice(g * EW, (g + 1) * EW)
        m = {
            "xT": xT[b],
            "cT": cT[b],
            "wq": np.ascontiguousarray(w_q[:, cols]).astype(bf16),
            "wk": np.ascontiguousarray(w_k[:, cols]).astype(bf16),
            "wv": np.ascontiguousarray(w_v[:, cols]).astype(bf16),
            "wck": np.ascontiguousarray(w_ctx_k[:, cols]).astype(bf16),
            "wcv": np.ascontiguousarray(w_ctx_v[:, cols]).astype(bf16),
            "wo": np.ascontiguousarray(w_out[:, cols]).astype(bf16),
            "qnw": q_norm_w.reshape(HD, 1).astype(np.float32).copy(),
            "knw": k_norm_w.reshape(HD, 1).astype(np.float32).copy(),
        }
        if with_mask:
            # mask [B,1,K,S] -> transposed [S,K] per batch (fp32).
            # The kernel folds the 1/sqrt(HD) score scale into the exp
            # activation, which would scale the mask too; pre-divide so
            # exp((scores_raw + mask/SCALE) * SCALE) = exp(scores + mask).
            m["maskT"] = np.ascontiguousarray(attn_mask[b, 0].T) * (1.0 / SCALE)
        in_maps.append(m)
    return in_maps



# revision 11
# speedup vs baseline: 1.0493x; 1.0493x over previous
"""Trainium2 Bass kernel for DFlashAttentionV5.

Reference computation (fp32, single device):
    Q/K/V/Kctx/Vctx projections -> rmsnorm(Q), rmsnorm(K_full) -> softmax
    attention over concat(ctx, self) keys/values -> output projection.

Sharding over 8 NeuronCores: batch (2-way) x head-group (4-way).
Core c handles batch b = c // 4 and heads 4*g..4*g+3 where g = c % 4.
Each core computes attention for its 4 local heads, then the transposed
full-width output-projection partial outT[2048 cols, 1024 tokens] (same
PE cost as a 512-col slice: the contraction shrinks to the 512 local E
dims), and a single ReduceScatter sums the partials across the 4-core
batch group, leaving each core its 512-row slice of outT. The collective
cost model charges ~15us fixed + out_bytes/40GBps per op, so one 1MB-out
ReduceScatter (~41us) beats four chained 1MB-out AllGathers (~165us).

All matmuls run in bf16 (fp32 PSUM accumulation); softmax statistics and
normalization factors are computed in fp32. x/ctx arrive pre-transposed
from the host ([D, tokens]) so no on-device transposes are needed.

Self-contained: hardcodes all shapes; only imports concourse + numpy.
"""

import math

import numpy as np
import ml_dtypes

import concourse.bass as bass
import concourse.mybir as mybir
import concourse.tile as tile
from concourse.bass_utils import run_bass_kernel_spmd

BF16 = mybir.dt.bfloat16
F32 = mybir.dt.float32
AF = mybir.ActivationFunctionType
ALU = mybir.AluOpType

# Problem dims
B, K, CTX, D, H, HD = 2, 1024, 2048, 2048, 16, 128
S = CTX + K            # 3072 keys per query
NCORES = 8
GROUPS = 4             # head groups (tensor-parallel within a batch)
NH = H // GROUPS       # 4 local heads per core
E = H * HD             # 2048
EW = NH * HD           # 512 local attention width / weight shard width
DCH = D // 128         # 16 contraction chunks
SCH = S // 128         # 24 key chunks
TCH = K // 128         # 8 query-token chunks
SCALE = 1.0 / math.sqrt(HD)
EPS = 1e-6
REPLICA_GROUPS = [[0, 1, 2, 3], [4, 5, 6, 7]]

_CACHE = {}


def _build(with_mask: bool):
    """Build the SPMD bass program (same program on all 8 cores)."""
    nc = bass.Bass(num_devices=NCORES)

    xT_d = nc.declare_dram_parameter("xT", [D, K], BF16, isOutput=False)
    cT_d = nc.declare_dram_parameter("cT", [D, CTX], BF16, isOutput=False)
    wq_d = nc.declare_dram_parameter("wq", [D, EW], BF16, isOutput=False)
    wk_d = nc.declare_dram_parameter("wk", [D, EW], BF16, isOutput=False)
    wv_d = nc.declare_dram_parameter("wv", [D, EW], BF16, isOutput=False)
    wck_d = nc.declare_dram_parameter("wck", [D, EW], BF16, isOutput=False)
    wcv_d = nc.declare_dram_parameter("wcv", [D, EW], BF16, isOutput=False)
    wo_d = nc.declare_dram_parameter("wo", [EW, E], BF16, isOutput=False)
    qnw_d = nc.declare_dram_parameter("qnw", [HD, 1], F32, isOutput=False)
    knw_d = nc.declare_dram_parameter("knw", [HD, 1], F32, isOutput=False)
    if with_mask:
        mt_d = nc.declare_dram_parameter("maskT", [S, K], F32, isOutput=False)
    # transposed output slice [out-cols 512, tokens 1024], host transposes
    out_d = nc.declare_dram_parameter("out", [EW, K], BF16, isOutput=True)

    with tile.TileContext(nc, num_cores=NCORES) as tc:
        with (
            tc.tile_pool(name="const", bufs=1) as constp,
            tc.tile_pool(name="perm", bufs=1) as perm,
            tc.tile_pool(name="stat", bufs=2) as statp,
            tc.tile_pool(name="bc", bufs=2) as bcp,
            tc.tile_pool(name="psA", bufs=3, space="PSUM") as psA,
            tc.tile_pool(name="ps1", bufs=2, space="PSUM") as ps1,
            tc.tile_pool(name="dram", bufs=1, space="DRAM") as dram,
        ):
            ones_col = constp.tile([128, 1], BF16)
            nc.any.memset(ones_col, 1.0)
            ones_row = constp.tile([1, 128], BF16)
            nc.any.memset(ones_row, 1.0)
            qnw_sb = constp.tile([HD, 1], F32)
            knw_sb = constp.tile([HD, 1], F32)

            # Resident tensors (bf16):
            #   K_sb[h]  [128=hd, 3072=s] per local head (ctx keys then self)
            #   V_sb[s]  [128=s-chunk, 512=4 heads x hd], s 0..15 ctx, 16..23 self
            #   QT_sb[h] [128=hd, 1024=q]
            K_sb = [perm.tile([128, S], BF16, tag=f"K{h}", bufs=1, name=f"K{h}")
                    for h in range(NH)]
            V_sb = [perm.tile([128, EW], BF16, tag=f"V{s}", bufs=1, name=f"V{s}")
                    for s in range(SCH)]
            QT_sb = [perm.tile([128, K], BF16, tag=f"Q{h}", bufs=1, name=f"Q{h}")
                     for h in range(NH)]
            attnT_sb = [perm.tile([128, K], BF16, tag=f"A{h}", bufs=1,
                                  name=f"A{h}") for h in range(NH)]

            # full-width transposed out-proj partial, reduce-scattered
            partial_d = dram.tile([E, K], BF16, name="partial")
            rs_out_d = dram.tile([EW, K], BF16, name="rsout")

            # ---- helper: rmsnorm in transposed layout.
            # ps [128=hd, width=tokens] fp32 PSUM -> dest bf16 SBUF.
            # norm over hd (partitions): mean of squares via ones-matmul,
            # rsqrt via reciprocal+sqrt, broadcast across partitions via K=1
            # matmul, apply with one scalar_tensor_tensor (folds norm weight).
            def rms_norm_T(sqp, ps, dest_ap, width, nw_sb):
                # norm over hd (partitions): mean of squares via ones-matmul,
                # rsqrt via reciprocal+sqrt, broadcast across partitions via
                # K=1 matmul, applied with one scalar_tensor_tensor (also
                # folds the norm weight).
                sqt = sqp.tile([128, 1024], BF16, tag="sq")
                nc.scalar.square(sqt[:, :width], ps[:, :width])
                for j in range(width // 512):
                    js = slice(j * 512, (j + 1) * 512)
                    ps_s = ps1.tile([128, 512], F32, tag="ps1")
                    nc.tensor.matmul(ps_s[0:1, :], ones_col[:], sqt[:, js],
                                     start=True, stop=True)
                    mean = statp.tile([1, 512], F32, tag="mean")
                    nc.vector.tensor_scalar(mean[:], ps_s[0:1, :], 1.0 / HD, EPS,
                                            ALU.mult, ALU.add)
                    rec = statp.tile([1, 512], F32, tag="rec")
                    nc.vector.reciprocal(rec[:], mean[:])
                    rs = statp.tile([1, 512], BF16, tag="rs")
                    nc.scalar.sqrt(rs[:], rec[:])  # rsqrt = sqrt(1/x), bf16
                    ps_b = ps1.tile([128, 512], F32, tag="ps1")
                    nc.tensor.matmul(ps_b[:], ones_row[:], rs[:],
                                     start=True, stop=True)
                    bc = bcp.tile([128, 512], F32, tag="bc")
                    nc.scalar.copy(bc[:], ps_b[:])
                    nc.vector.scalar_tensor_tensor(
                        dest_ap[:, js], ps[:, js], nw_sb[:], bc[:],
                        ALU.mult, ALU.mult)

            # ================= projection phase =================
            with (
                tc.tile_pool(name="srcT", bufs=1) as srcTp,
                tc.tile_pool(name="wstream", bufs=4) as wstream,
                tc.tile_pool(name="wwide", bufs=1) as wwide,
                tc.tile_pool(name="sqp", bufs=2) as sqp,
            ):
                # d-chunk accessor over grouped source tiles [128, 4*1024]
                def src_at(grp, d):
                    return grp[d // 4], (d % 4) * 1024

                def load_set(grp, dram_ap, split_first=False):
                    # dram_ap: [D, 1024] (d-major); one DMA per 4 d-chunks.
                    # split_first peels d-chunk 0 into its own small DMA so
                    # the first dependent matmul can start sooner.
                    if split_first:
                        nc.sync.dma_start(
                            grp[0][:, 0:1024],
                            dram_ap[0:128, :])
                        nc.sync.dma_start(
                            grp[0][:, 1024:4096].rearrange("p (a t) -> p a t",
                                                           t=1024),
                            dram_ap[128:512, :]
                            .rearrange("(a p) t -> p a t", p=128))
                    else:
                        nc.sync.dma_start(
                            grp[0][:].rearrange("p (a t) -> p a t", t=1024),
                            dram_ap[0:512, :]
                            .rearrange("(a p) t -> p a t", p=128))
                    for i in range(1, 4):
                        nc.sync.dma_start(
                            grp[i][:].rearrange("p (a t) -> p a t", t=1024),
                            dram_ap[i * 512:(i + 1) * 512, :]
                            .rearrange("(a p) t -> p a t", p=128))

                # Q^T / K^T projections (weight-stationary):
                # psum[c] [128=col-chunk(head), 1024 tokens] += w[d,c].T @ srcT[d]
                def load_wchunk(w_d, c, name=None):
                    wch = wstream.tile([128, D], BF16, tag="w", name=name)
                    nc.sync.dma_start(
                        wch[:].rearrange("p (a q) -> p a q", q=128),
                        w_d[:, c * 128:(c + 1) * 128]
                        .rearrange("(a p) q -> p a q", p=128))
                    return wch

                def qk_proj(w_d, srcT, dest_of_chunk, nw_sb, pre=None):
                    for c in range(EW // 128):
                        wch = pre if (c == 0 and pre is not None) \
                            else load_wchunk(w_d, c)
                        ps = psA.tile([128, 1024], F32, tag="psA",
                                      name=f"psqk{c}")
                        for d in range(DCH):
                            st, off = src_at(srcT, d)
                            for j in range(2):
                                nc.tensor.matmul(
                                    ps[:, j * 512:(j + 1) * 512],
                                    wch[:, d * 128:(d + 1) * 128],
                                    st[:, off + j * 512:off + j * 512 + 512],
                                    start=(d == 0), stop=(d == DCH - 1))
                        dest, off = dest_of_chunk(c)
                        rms_norm_T(sqp, ps, dest[:, off:off + 1024], 1024, nw_sb)

                # V projections (activation-stationary):
                # V_sb[s] [128=tokens, 512=cols] += srcT[d][:,t-chunk].T @ wv[d]
                def load_wide(w_d, tag):
                    grp = []
                    for i in range(4):
                        wt = wwide.tile([128, 4 * EW], BF16, tag=f"wv{i}",
                                        bufs=1, name=f"wv{i}_{tag}")
                        nc.sync.dma_start(
                            wt[:].rearrange("p (a q) -> p a q", q=EW),
                            w_d[i * 512:(i + 1) * 512, :]
                            .rearrange("(a p) q -> p a q", p=128))
                        grp.append(wt)
                    return grp

                def v_proj(wv_grp, srcT, s_base):
                    for t in range(TCH):
                        ps = ps1.tile([128, 512], F32, tag="ps1")
                        for d in range(DCH):
                            st, off = src_at(srcT, d)
                            wvt = wv_grp[d // 4]
                            wo_off = (d % 4) * EW
                            nc.tensor.matmul(
                                ps[:], st[:, off + t * 128:off + (t + 1) * 128],
                                wvt[:, wo_off:wo_off + EW],
                                start=(d == 0), stop=(d == DCH - 1))
                        nc.vector.tensor_copy(V_sb[s_base + t][:], ps[:])

                # slot sets: A = xT then ctx-half1 (recycled), B = ctx-half0
                setA = [srcTp.tile([128, 4096], BF16, tag=f"sa{i}", bufs=1,
                                   name=f"xT{i}") for i in range(4)]
                setB = [srcTp.tile([128, 4096], BF16, tag=f"sb{i}", bufs=1,
                                   name=f"cTa{i}") for i in range(4)]
                pre_wq = load_wchunk(wq_d, 0, name="prewq")
                load_set(setA, xT_d, split_first=True)
                nc.sync.dma_start(qnw_sb[:], qnw_d[:])
                nc.sync.dma_start(knw_sb[:], knw_d[:])

                # self tokens (block A): Q, K_self, V_self
                qk_proj(wq_d, setA, lambda c: (QT_sb[c], 0), qnw_sb, pre=pre_wq)
                load_set(setB, cT_d[:, 0:1024])
                wv_sb = load_wide(wv_d, "s")
                qk_proj(wk_d, setA, lambda c: (K_sb[c], CTX), knw_sb)
                v_proj(wv_sb, setA, CTX // 128)

                # ctx half 0 (block B): K_ctx[:, 0:1024], V_ctx s-chunks 0..7
                wcv_sb = load_wide(wcv_d, "c0")
                qk_proj(wck_d, setB, lambda c: (K_sb[c], 0), knw_sb)
                v_proj(wcv_sb, setB, 0)

                # ctx half 1 reuses set A slots
                setC = [srcTp.tile([128, 4096], BF16, tag=f"sa{i}", bufs=1,
                                   name=f"cTb{i}") for i in range(4)]
                load_set(setC, cT_d[:, 1024:2048])
                wcv2_sb = load_wide(wcv_d, "c1")
                qk_proj(wck_d, setC, lambda c: (K_sb[c], 1024), knw_sb)
                v_proj(wcv2_sb, setC, TCH)

            # ================= attention + output phase =================
            with (
                tc.tile_pool(name="probsT", bufs=8) as probsp,
                tc.tile_pool(name="dacc", bufs=2) as daccp,
                tc.tile_pool(name="accb", bufs=2) as accbp,
                tc.tile_pool(name="wop", bufs=1) as wop,
                tc.tile_pool(name="ostg", bufs=3) as ostgp,
                tc.tile_pool(name="mrow", bufs=4) as mrowp,
            ):
                # w_out rows for local head h (global rows (g*NH+h)*128..),
                # all 2048 columns: [128=hd, 2048=out-cols]
                wo_sb = [wop.tile([128, E], BF16, tag=f"wo{h}", bufs=1,
                                  name=f"wo{h}") for h in range(NH)]
                for h in range(NH):
                    nc.sync.dma_start(wo_sb[h][:],
                                      wo_d[h * 128:(h + 1) * 128, :])

                # attention for one local head, transposed scores:
                # scoresT[s-chunk] [128=s, 1024=q] = K_chunk @ Q^T  (no max
                # subtraction: scores ~ N(0,1) after rmsnorm + 1/sqrt(HD))
                def attention(h, q0=0, qw=K, dst=None):
                    # processes queries [q0, q0+qw) for local head h
                    nj = qw // 512
                    ps_pv = psA.tile([128, 1024], F32, tag="psA", name=f"pv{h}_{q0}")
                    acc = daccp.tile([128, 1024], F32, tag="dacc", name=f"dac{h}")
                    for s in range(SCH):
                        ps_sT = psA.tile([128, 1024], F32, tag="psA",
                                         name=f"sT{h}_{q0}_{s}")
                        for j in range(nj):
                            nc.tensor.matmul(
                                ps_sT[:, j * 512:(j + 1) * 512],
                                K_sb[h][:, s * 128:(s + 1) * 128],
                                QT_sb[h][:, q0 + j * 512:q0 + (j + 1) * 512],
                                start=True, stop=True)
                        if with_mask:
                            mrow = mrowp.tile([128, K], F32, tag="mrow")
                            nc.sync.dma_start(
                                mrow[:, :qw],
                                mt_d[s * 128:(s + 1) * 128, q0:q0 + qw])
                            nc.vector.tensor_tensor(ps_sT[:, :qw], ps_sT[:, :qw],
                                                    mrow[:, :qw], ALU.add)
                        pT = probsp.tile([128, 1024], BF16, tag="pT")
                        nc.scalar.activation(pT[:, :qw], ps_sT[:, :qw], AF.Exp,
                                             scale=SCALE)
                        first, last = (s == 0), (s == SCH - 1)
                        # probs accumulate on the DVE (f32) for the softmax
                        # denominators; the PE only does scores and PV
                        if first:
                            nc.vector.tensor_copy(acc[:, :qw], pT[:, :qw])
                        else:
                            nc.vector.tensor_tensor(acc[:, :qw], acc[:, :qw],
                                                    pT[:, :qw], ALU.add)
                        for j in range(nj):
                            js = slice(j * 512, (j + 1) * 512)
                            nc.tensor.matmul(
                                ps_pv[:, js],
                                V_sb[s][:, h * 128:(h + 1) * 128], pT[:, js],
                                start=first, stop=last)
                    # normalize: attnT = ps_pv * (1/denom), denom broadcast
                    # across partitions via K=1 matmul
                    at = dst if dst is not None else attnT_sb[h]
                    accb = accbp.tile([128, 1024], BF16, tag="accb")
                    nc.vector.tensor_copy(accb[:, :qw], acc[:, :qw])
                    for j in range(nj):
                        js = slice(j * 512, (j + 1) * 512)
                        ps_d = ps1.tile([128, 512], F32, tag="ps1")
                        nc.tensor.matmul(ps_d[0:1, :], ones_col[:], accb[:, js],
                                         start=True, stop=True)
                        rec = statp.tile([1, 512], F32, tag="rec")
                        nc.vector.reciprocal(rec[:], ps_d[0:1, :])
                        rb = statp.tile([1, 512], BF16, tag="rb")
                        nc.vector.tensor_copy(rb[:], rec[:])
                        ps_b = ps1.tile([128, 512], F32, tag="ps1")
                        nc.tensor.matmul(ps_b[:], ones_row[:], rb[:],
                                         start=True, stop=True)
                        bc = bcp.tile([128, 512], F32, tag="bc")
                        nc.scalar.copy(bc[:], ps_b[:])
                        nc.vector.tensor_tensor(at[:, js], ps_pv[:, js], bc[:],
                                                ALU.mult)

                for h in range(NH):
                    attention(h)

                # transposed out-proj partial: for each 128-col chunk c of E,
                # outT[c][128 cols, 1024 tok] = sum_h wo_sb[h][:,c].T @ attnT[h]
                # (PSUM-accumulated over the 4 local heads), staged to bf16 and
                # DMAd into partial_d; one ReduceScatter then sums partials
                # across the 4-core group, leaving rows [g*512,(g+1)*512) =
                # this core's transposed output slice.
                for c in range(E // 128):
                    ps = psA.tile([128, 1024], F32, tag="psA", name=f"ot{c}")
                    for h in range(NH):
                        for j in range(2):
                            js = slice(j * 512, (j + 1) * 512)
                            nc.tensor.matmul(
                                ps[:, js], wo_sb[h][:, c * 128:(c + 1) * 128],
                                attnT_sb[h][:, js],
                                start=(h == 0), stop=(h == NH - 1))
                    stg = ostgp.tile([128, K], BF16, tag="ostg")
                    nc.vector.tensor_copy(stg[:], ps[:])
                    nc.sync.dma_start(partial_d[c * 128:(c + 1) * 128, :],
                                      stg[:])

                nc.gpsimd.collective_compute(
                    "ReduceScatter", ALU.add,
                    replica_groups=REPLICA_GROUPS,
                    ins=[partial_d[:].opt()],
                    outs=[rs_out_d[:].opt()],
                )
                nc.sync.dma_start(out_d[:], rs_out_d[:])

    return nc


def _split_multiwaits(nc):
    """walrus codegen in this container rejects instructions with more than
    one semaphore wait; split the excess onto preceding NoOps on the same
    engine."""
    for f in nc.m.functions:
        for blk in f.blocks:
            idx = 0
            while idx < len(blk.instructions):
                inst = blk.instructions[idx]
                si = inst.sync_info
                maxw = 1
                if si is None or len(si.on_wait) <= maxw:
                    idx += 1
                    continue
                waits = list(si.on_wait)
                ncarry = (len(waits) - 1) // maxw  # leave <=maxw on inst
                for k in range(ncarry):
                    chunk = waits[k * maxw:(k + 1) * maxw]
                    nop = mybir.InstNoOp(
                        name=nc.get_next_instruction_name(),
                        ins=[], outs=[],
                        bass_nofuse=True,
                        sync_info=mybir.SyncInfo(on_wait=chunk, on_update=[]),
                    )
                    nop.engine = inst.engine
                    nc.register_instruction(nop)
                    blk.instructions.insert(idx, nop)
                    idx += 1
                si.on_wait = waits[ncarry * maxw:]
                idx += 1


def _get_program(with_mask: bool):
    key = ("prog", with_mask)
    if key not in _CACHE:
        nc = _build(with_mask)
        _split_multiwaits(nc)
        _CACHE[key] = nc
    return _CACHE[key]


def kernel(x, context, attn_mask, w_q, w_k, w_v, w_ctx_k, w_ctx_v, w_out,
           q_norm_w, k_norm_w):
    x = np.asarray(x, np.float32)
    context = np.asarray(context, np.float32)
    attn_mask = np.asarray(attn_mask, np.float32)
    w_q = np.asarray(w_q, np.float32)
    w_k = np.asarray(w_k, np.float32)
    w_v = np.asarray(w_v, np.float32)
    w_ctx_k = np.asarray(w_ctx_k, np.float32)
    w_ctx_v = np.asarray(w_ctx_v, np.float32)
    w_out = np.asarray(w_out, np.float32)
    q_norm_w = np.asarray(q_norm_w, np.float32)
    k_norm_w = np.asarray(k_norm_w, np.float32)

    with_mask = bool(np.any(attn_mask))
    nc = _get_program(with_mask)
    in_maps = _prepare_in_maps(x, context, attn_mask, w_q, w_k, w_v, w_ctx_k,
                               w_ctx_v, w_out, q_norm_w, k_norm_w, with_mask)

    res = run_bass_kernel_spmd(nc, in_maps, list(range(NCORES))).results
    return _assemble(res)


def _assemble(res):
    out = np.empty((B, K, D), np.float32)
    for c in range(NCORES):
        b, g = c // GROUPS, c % GROUPS
        out[b, :, g * EW:(g + 1) * EW] = res[c]["out"].astype(np.float32).T
    return out


def _prepare_in_maps(x, context, attn_mask, w_q, w_k, w_v, w_ctx_k, w_ctx_v,
                     w_out, q_norm_w, k_norm_w, with_mask):
    bf16 = ml_dtypes.bfloat16
    xT = [np.ascontiguousarray(x[b].T).astype(bf16) for b in range(B)]
    cT = [np.ascontiguousarray(context[b].T).astype(bf16) for b in range(B)]
    in_maps = []
    for c in range(NCORES):
        b, g = c // GROUPS, c % GROUPS
        cols = slice(g * EW, (g + 1) * EW)
        m = {
            "xT": xT[b],
            "cT": cT[b],
            "wq": np.ascontiguousarray(w_q[:, cols]).astype(bf16),
            "wk": np.ascontiguousarray(w_k[:, cols]).astype(bf16),
            "wv": np.ascontiguousarray(w_v[:, cols]).astype(bf16),
            "wck": np.ascontiguousarray(w_ctx_k[:, cols]).astype(bf16),
            "wcv": np.ascontiguousarray(w_ctx_v[:, cols]).astype(bf16),
            "wo": np.ascontiguousarray(w_out[g * EW:(g + 1) * EW, :])
            .astype(bf16),
            "qnw": q_norm_w.reshape(HD, 1).astype(np.float32).copy(),
            "knw": k_norm_w.reshape(HD, 1).astype(np.float32).copy(),
        }
        if with_mask:
            # mask [B,1,K,S] -> transposed [S,K] per batch (fp32).
            # The kernel folds the 1/sqrt(HD) score scale into the exp
            # activation, which would scale the mask too; pre-divide so
            # exp((scores_raw + mask/SCALE) * SCALE) = exp(scores + mask).
            m["maskT"] = np.ascontiguousarray(attn_mask[b, 0].T) * (1.0 / SCALE)
        in_maps.append(m)
    return in_maps

